# revision 32
# baseline (speedup 1.0000x reference)
"""Trainium2 Bass kernel for BucketingBBoxCoder (nms_detection).

Per proposal and per side (l,r,t,d): softmax over 7 bucket logits, top-2
values+indices, offset gather at top-1 index, bbox decode + clip, and a
location confidence score.

Sharding: N split across 8 cores (embarrassingly parallel).

Engine split (per tile):
  ACT : exp, Sign (second-max mask), Abs, scalar scalings
  DVE : grouped reduces (X-axis), is_equal mask, mask chains (stt)
  Pool: elementwise add/sub/mult passes, clips
"""

import sys

if "/opt/trn_rl_repo" not in sys.path:
    sys.path.insert(0, "/opt/trn_rl_repo")

import numpy as np

import concourse.bass as bass
import concourse.bacc as bacc
import concourse.mybir as mybir
import concourse.tile as tile
from concourse.bass_utils import run_bass_kernel_spmd

B = 8
N = 131072
SIDE = 7
R = 4 * SIDE
NCORES = 8
NS = N // NCORES
M = B * NS                # proposals per core
MAX_W = 1333.0 - 1.0
MAX_H = 800.0 - 1.0
SCALE = 3.0
NB = 14.0

P = 128
T = 64                    # proposals per partition per tile
TILE_PROPS = P * T
NT = M // TILE_PROPS

F32 = mybir.dt.float32
AX = mybir.AxisListType.X
OP = mybir.AluOpType
AF = mybir.ActivationFunctionType

_BUILT = None
_YSEL = None


def _register_ysel():
    """Custom DVE op: out = (Src0 == Src1) ? -FLT_MAX : Src0 — masks the
    top-1 slot in one pass (replaces is_equal + scalar_tensor_tensor)."""
    global _YSEL
    if _YSEL is not None:
        return _YSEL
    import concourse.dve_ops as dvo
    from concourse.dve_spec import Spec, Src0, Src1, MaxNeg, eq, select, lower
    from concourse.dve_uop import DveOpSpec

    name = "ANT_Y_SELECT"
    for o in dvo.OPS:
        if o.name == name:
            _YSEL = o
            return o
    spec = Spec(
        body=select(eq(Src0, Src1), MaxNeg, Src0),
        reference=lambda in0, in1, s0, s1, imm2: np.where(
            in0 == in1, np.float32(-3.4028235e38), in0),
    )
    opc = max(dvo._SUB_OPCODE_FOR_NAME.values()) + 1
    assert opc < 0x20
    dvo._SUB_OPCODE_FOR_NAME[name] = opc
    shas = {}
    for ver in ("v3", "v4"):
        try:
            u = lower(spec, ver=ver)
            shas[ver] = DveOpSpec(name=name, opcode=opc, uops=u,
                                  rd1_en=True).sha(ver)
        except Exception:
            pass
    op = dvo.DveOp(name, spec, subdim=False, uops_sha=shas)
    dvo.OPS.append(op)
    dvo.CUSTOM_DVE_SPECS[name] = spec
    _YSEL = op
    return op


def _build():
    global _BUILT
    if _BUILT is not None:
        return _BUILT

    nc = bacc.Bacc(None, target_bir_lowering=False)

    # const AP for the Abs bias (-21): same pattern as Bass.__init__
    _c = nc.alloc_sbuf_tensor("const-float32-neg21", [128, 1], F32)
    nc.gpsimd.memset(_c.ap(), -21.0)
    nc.const_aps.aps[(F32, -21.0)] = _c.ap()
    nc.all_engine_barrier()

    cls_d = nc.dram_tensor("cls", [M, R], F32, kind="ExternalInput")
    off_d = nc.dram_tensor("off", [M, R], F32, kind="ExternalInput")
    prp_d = nc.dram_tensor("prp", [M, 4], F32, kind="ExternalInput")
    bbx_d = nc.dram_tensor("bbx", [M, 4], F32, kind="ExternalOutput")
    cnf_d = nc.dram_tensor("cnf", [M], F32, kind="ExternalOutput")

    cls_r = cls_d[:, :].rearrange("(i p t) r -> i p (t r)", p=P, t=T)
    off_r = off_d[:, :].rearrange("(i p t) r -> i p (t r)", p=P, t=T)
    prp_r = prp_d[:, :].rearrange("(i p t) r -> i p (t r)", p=P, t=T)
    bbx_r = bbx_d[:, :].rearrange("(i p t) r -> i p (t r)", p=P, t=T)
    cnf_r = cnf_d[:].rearrange("(i p t) -> i p t", p=P, t=T)

    with tile.TileContext(nc) as tc:
        with tc.tile_pool(name="bigio", bufs=3) as bigio, \
             tc.tile_pool(name="big", bufs=2) as big, \
             tc.tile_pool(name="small", bufs=2) as small, \
             tc.tile_pool(name="cgp", bufs=3) as cgp:
            def stage_a(i):
                h = {}
                cls_t = bigio.tile([P, T * R], F32, tag="cls")
                off_t = bigio.tile([P, T * R], F32, tag="off")
                prp_t = small.tile([P, T * 4], F32, tag="prp")
                nc.sync.dma_start(out=cls_t[:], in_=cls_r[i])
                nc.sync.dma_start(out=off_t[:], in_=off_r[i])
                nc.sync.dma_start(out=prp_t[:], in_=prp_r[i])

                cls3 = cls_t[:].rearrange("p (n s) -> p n s", s=SIDE)
                off3 = off_t[:].rearrange("p (n s) -> p n s", s=SIDE)

                # E = exp(cls) into first half of eo (om goes in the
                # second half so Z and osel reduce in ONE instruction)
                eo_t = big.tile([P, 2 * T * R], F32, tag="eo")
                nc.scalar.activation(eo_t[:, 0:T * R], cls_t[:], AF.Exp)
                h["eo_t"] = eo_t

                # top-1 on raw logits (same ordering as softmax)
                m1_t = small.tile([P, T * 4], F32, tag="m1")
                nc.vector.reduce_max(m1_t[:], cls3, axis=AX)
                m1b = m1_t[:].broadcast_to((P, T * 4, SIDE))
                mk_t = big.tile([P, 2 * T * R], F32, tag="mk")
                eq_h = mk_t[:, 0:T * R]
                eq3 = eq_h.rearrange("p (n s) -> p n s", s=SIDE)

                nc.vector.tensor_tensor(eq3, cls3, m1b, OP.is_equal)

                # gather mask product early so Pool can run ahead
                om3 = eo_t[:, T * R:].rearrange("p (n s) -> p n s", s=SIDE)
                nc.gpsimd.tensor_tensor(om3, eq3, off3, OP.mult)

                # Y = cls - 1e30*eq1; M2 = max_s Y  (second max)
                y_t = big.tile([P, T * R], F32, tag="y")
                nc.vector.scalar_tensor_tensor(
                    y_t[:], eq_h, -1.0e30, cls_t[:], OP.mult, OP.add)
                y3 = y_t[:].rearrange("p (n s) -> p n s", s=SIDE)
                m2_t = small.tile([P, T * 4], F32, tag="m2")
                nc.vector.reduce_max(m2_t[:], y3, axis=AX)

                # second-index mask via sign: sgn2 = sign(Y - M2) in {-1, 0}
                m2b = m2_t[:].broadcast_to((P, T * 4, SIDE))
                df2_t = big.tile([P, T * R], F32, tag="df2")
                df23 = df2_t[:].rearrange("p (n s) -> p n s", s=SIDE)
                nc.gpsimd.tensor_tensor(df23, y3, m2b, OP.subtract)
                nc.scalar.activation(mk_t[:, T * R:], df2_t[:], AF.Sign)
                h["mk_t"] = mk_t

                # score values + 1/Z
                e1_t = small.tile([P, T * 4], F32, tag="e1")
                nc.scalar.activation(e1_t[:], m1_t[:], AF.Exp)
                e2_t = small.tile([P, T * 4], F32, tag="e2")
                nc.scalar.activation(e2_t[:], m2_t[:], AF.Exp)
                h.update(e1_t=e1_t, e2_t=e2_t)

                # ---- bbox geometry (independent of cls chain) ----
                x1 = prp_t[:, 0::4]
                y1 = prp_t[:, 1::4]
                x2 = prp_t[:, 2::4]
                y2 = prp_t[:, 3::4]
                wd_t = small.tile([P, T], F32, tag="wd")
                nc.gpsimd.tensor_tensor(wd_t[:], x2, x1, OP.subtract)
                ht_t = small.tile([P, T], F32, tag="ht")
                nc.gpsimd.tensor_tensor(ht_t[:], y2, y1, OP.subtract)

                # NOTE: geometry tiles below use slot order (l,t,r,d)
                # (= proposal coord order x1,y1,x2,y2) instead of (l,r,t,d)
                bs_t = small.tile([P, T * 4], F32, tag="bs")
                nc.scalar.mul(bs_t[:, 0::4], wd_t[:], SCALE / NB)
                nc.scalar.mul(bs_t[:, 1::4], ht_t[:], SCALE / NB)
                nc.scalar.mul(bs_t[:, 2::4], wd_t[:], -SCALE / NB)
                nc.scalar.mul(bs_t[:, 3::4], ht_t[:], -SCALE / NB)
                bu_t = small.tile([P, T * 4], F32, tag="bu")
                nc.scalar.mul(bu_t[:, 0::4], wd_t[:], SCALE / NB)
                nc.scalar.mul(bu_t[:, 1::4], ht_t[:], SCALE / NB)
                nc.scalar.mul(bu_t[:, 2::4], wd_t[:], SCALE / NB)
                nc.scalar.mul(bu_t[:, 3::4], ht_t[:], SCALE / NB)
                hs_t = small.tile([P, T * 4], F32, tag="hs")
                nc.scalar.mul(hs_t[:, 0::4], wd_t[:], SCALE / (2 * NB))
                nc.scalar.mul(hs_t[:, 1::4], ht_t[:], SCALE / (2 * NB))
                nc.scalar.mul(hs_t[:, 2::4], wd_t[:], -SCALE / (2 * NB))
                nc.scalar.mul(hs_t[:, 3::4], ht_t[:], -SCALE / (2 * NB))

                # px_j = 2*prp_j - prp_{j xor 2} for all 4 coords in ONE op:
                # partner view swaps the (x1,y1)/(x2,y2) halves via a
                # reversed middle dim
                prp3 = prp_t[:].rearrange("p (t g) -> p t g", g=4)
                px_t = small.tile([P, T * 4], F32, tag="px")
                px3 = px_t[:].rearrange("p (t g) -> p t g", g=4)
                nc.vector.scalar_tensor_tensor(
                    px3[:, :, 0:2], prp3[:, :, 0:2], 2.0, prp3[:, :, 2:4],
                    OP.mult, OP.subtract)
                nc.vector.scalar_tensor_tensor(
                    px3[:, :, 2:4], prp3[:, :, 2:4], 2.0, prp3[:, :, 0:2],
                    OP.mult, OP.subtract)
                pxh_t = small.tile([P, T * 4], F32, tag="pxh")
                nc.gpsimd.tensor_tensor(pxh_t[:], px_t[:], hs_t[:], OP.add)
                h.update(bs_t=bs_t, bu_t=bu_t, pxh_t=pxh_t)
                return h

            def stage_b(i, h):
                mk_t = h["mk_t"]
                # [Z | osel] in one grouped reduce over the eo halves
                eo3 = h["eo_t"][:].rearrange("p (q s) -> p q s", s=SIDE)
                zo_t = small.tile([P, 2 * T * 4], F32, tag="zo")
                nc.vector.reduce_sum(zo_t[:], eo3, axis=AX)
                z_t = zo_t[:, 0:T * 4]
                osel_t = zo_t[:, T * 4:]
                zi_t = small.tile([P, T * 4], F32, tag="zi")
                nc.vector.reciprocal(zi_t[:], z_t)

                # fused index chains: halves of mk are (eq1 | sgn2);
                # one stt per s computes [i1 | i2''] together
                mk3 = mk_t[:].rearrange("p (hh q) -> p hh q", hh=2)
                i12_t = small.tile([P, 2 * T * 4], F32, tag="i12")
                i12_3 = i12_t[:].rearrange("p (hh q) -> p hh q", hh=2)
                nc.vector.scalar_tensor_tensor(
                    i12_3, mk3[:, :, 2::SIDE], 2.0, mk3[:, :, 1::SIDE],
                    OP.mult, OP.add)
                for s in range(3, SIDE):
                    nc.vector.scalar_tensor_tensor(
                        i12_3, mk3[:, :, s::SIDE], float(s), i12_3,
                        OP.mult, OP.add)
                i1_t = i12_t[:, 0:T * 4]
                i2_t = i12_t[:, T * 4:]
                # dlt_true = i1 - (21 + i2''); ad = |i1 - i2'' - 21| via bias
                dr_t = small.tile([P, T * 4], F32, tag="dr")
                nc.gpsimd.tensor_tensor(dr_t[:], i1_t, i2_t, OP.subtract)
                ad_t = small.tile([P, T * 4], F32, tag="ad")
                nc.scalar.activation(ad_t[:], dr_t[:], AF.Abs, bias=-21.0)
                # conf_g = (e1 - e2 + e2*|dlt|) / Z
                u_t = small.tile([P, T * 4], F32, tag="u")
                nc.gpsimd.tensor_tensor(u_t[:], h["e1_t"][:], h["e2_t"][:], OP.subtract)
                v_t = small.tile([P, T * 4], F32, tag="v")
                nc.gpsimd.tensor_tensor(v_t[:], h["e2_t"][:], ad_t[:], OP.mult)
                w_t = small.tile([P, T * 4], F32, tag="w")
                nc.gpsimd.tensor_tensor(w_t[:], u_t[:], v_t[:], OP.add)
                cg_t = cgp.tile([P, T * 4], F32, tag="cg")
                nc.gpsimd.tensor_tensor(cg_t[:], w_t[:], zi_t[:], OP.mult)
                hb = {"cg_t": cg_t}

                # out = pxh + i1*bs - osel*bu  (geometry is in (l,t,r,d)
                # slot order; read i1/osel through a permuted view: group
                # sequence (0,2,1,3) via dims [2(stride 1), 2(stride 2)])
                mq_t = small.tile([P, T * 4], F32, tag="mq")
                mq4 = mq_t[:].rearrange("p (t g) -> p t g", g=4)
                bs4 = h["bs_t"][:].rearrange("p (t g) -> p t g", g=4)
                nc.gpsimd.tensor_tensor(
                    mq4.rearrange("p t (u v) -> p t u v", u=2),
                    i1_t.rearrange("p (t v u) -> p t u v", u=2, v=2),
                    bs4.rearrange("p t (u v) -> p t u v", u=2), OP.mult)
                oq_t = small.tile([P, T * 4], F32, tag="oq")
                oq4 = oq_t[:].rearrange("p (t g) -> p t g", g=4)
                bu4 = h["bu_t"][:].rearrange("p (t g) -> p t g", g=4)
                nc.gpsimd.tensor_tensor(
                    oq4.rearrange("p t (u v) -> p t u v", u=2),
                    osel_t.rearrange("p (t v u) -> p t u v", u=2, v=2),
                    bu4.rearrange("p t (u v) -> p t u v", u=2), OP.mult)
                bq_t = small.tile([P, T * 4], F32, tag="bq")
                nc.gpsimd.tensor_tensor(bq_t[:], h["pxh_t"][:], mq_t[:], OP.add)
                bb_t = small.tile([P, T * 4], F32, tag="bb")
                nc.gpsimd.tensor_tensor(bb_t[:], bq_t[:], oq_t[:], OP.subtract)

                # bb slots are (l,t,r,d) == output coord order (x1,y1,x2,y2)
                bbo_t = small.tile([P, T * 4], F32, tag="bbo")
                bb3 = bb_t[:].rearrange("p (t g) -> p t g", g=4)
                bbo3 = bbo_t[:].rearrange("p (t g) -> p t g", g=4)
                nc.gpsimd.tensor_scalar(
                    bbo3[:, :, 0:4:2], bb3[:, :, 0:4:2], 0.0, MAX_W, OP.max, OP.min)
                nc.gpsimd.tensor_scalar(
                    bbo3[:, :, 1:4:2], bb3[:, :, 1:4:2], 0.0, MAX_H, OP.max, OP.min)
                nc.sync.dma_start(out=bbx_r[i], in_=bbo_t[:])
                return hb

            def stage_c(i, hb):
                cg = hb["cg_t"]
                c01_t = small.tile([P, T], F32, tag="c01")
                nc.gpsimd.tensor_tensor(c01_t[:], cg[:, 0::4], cg[:, 1::4], OP.add)
                c23_t = small.tile([P, T], F32, tag="c23")
                nc.gpsimd.tensor_tensor(c23_t[:], cg[:, 2::4], cg[:, 3::4], OP.add)
                cf_t = small.tile([P, T], F32, tag="cf")
                nc.gpsimd.tensor_tensor(cf_t[:], c01_t[:], c23_t[:], OP.add)
                cfo_t = small.tile([P, T], F32, tag="cfo")
                nc.scalar.mul(cfo_t[:], cf_t[:], 0.25)
                nc.sync.dma_start(out=cnf_r[i], in_=cfo_t[:])

            ha_prev = None
            hb_prev = None
            for i in range(NT):
                ha = stage_a(i)
                hb = stage_b(i - 1, ha_prev) if ha_prev is not None else None
                if hb_prev is not None:
                    stage_c(i - 2, hb_prev)
                ha_prev, hb_prev = ha, hb
            hb = stage_b(NT - 1, ha_prev)
            stage_c(NT - 2, hb_prev)
            stage_c(NT - 1, hb)

    nc.compile()
    _BUILT = nc
    return nc


def kernel(proposals, cls_preds, offset_preds):
    proposals = np.ascontiguousarray(np.asarray(proposals, dtype=np.float32))
    cls_preds = np.ascontiguousarray(np.asarray(cls_preds, dtype=np.float32))
    offset_preds = np.ascontiguousarray(np.asarray(offset_preds, dtype=np.float32))

    cls3 = cls_preds.reshape(B, N, R)
    off3 = offset_preds.reshape(B, N, R)

    in_maps = []
    for k in range(NCORES):
        sl = slice(k * NS, (k + 1) * NS)
        in_maps.append({
            "cls": np.ascontiguousarray(cls3[:, sl].reshape(M, R)),
            "off": np.ascontiguousarray(off3[:, sl].reshape(M, R)),
            "prp": np.ascontiguousarray(proposals[:, sl].reshape(M, 4)),
        })

    nc = _build()
    # first execution of a fresh NEFF occasionally dies with a transient
    # NRT_EXEC_UNIT_UNRECOVERABLE; a retry has always succeeded
    try:
        res = run_bass_kernel_spmd(nc, in_maps, list(range(NCORES)))
    except Exception:
        res = run_bass_kernel_spmd(nc, in_maps, list(range(NCORES)))

    bboxes = np.empty((B, N, 4), dtype=np.float32)
    conf = np.empty((B, N), dtype=np.float32)
    for k in range(NCORES):
        sl = slice(k * NS, (k + 1) * NS)
        bboxes[:, sl] = res.results[k]["bbx"].reshape(B, NS, 4)
        conf[:, sl] = res.results[k]["cnf"].reshape(B, NS)
    return bboxes, conf
